# revision 3
# baseline (speedup 1.0000x reference)
"""Trainium2 Bass kernel for the gated two-path (semantic+RoPE-geometric) causal
attention layer.  8-core sharding: 2 heads x BOTH batches per core, with
on-device collectives so every unique input byte crosses PJRT exactly once.

Reference computation (B=2, S=2048, D_MODEL=2048, H=16, DS=DG=64, DV=128):
  qs=x@wq_sem, ks=x@wk_sem, qg=rope(x@wq_geo), kg=rope(x@wk_geo), v=x@wv
  scores = g*qs.ks/8 + (1-g)*qg.kg/8 ; causal softmax ; out=(attn@v)@wo

Data distribution (everything bf16 on the wire, 2 collectives total):
  - ONE 8-rank AllGather rebuilds a shared blob on every core from 1/8
    slices: xT for both batches, rope trig tables (f16 bits), the causal
    staircase mask, and the wo projection (all identical-per-core data)
  - wq/wk/wv slices for the core's 2 heads ship directly (distinct per core)
  - after attention, an 8-rank AllToAll reshards from (2 heads, all tokens)
    to (all 16 heads, my 512-token eighth); each core then computes the
    full output projection for its eighth -> out [DM, 512] bf16

Per-core compute (a "slot" is a (batch, local head) pair):
  - host folds sigmoid(gate)/sqrt(d) into wq and concatenates [sem|geo]
    per head so each head's QK^T is one K=128 contraction
  - projections: qcatT/kcatT [128, slot, S] via lhsT=weight tiles,
    rhs=xT chunks from the blob; rope fused per 512-token slice
  - scores^T per k-tile; causal staircase mask added on DVE into PSUM
  - exp on ScalarE (no max subtraction; |scores| <~ 8 << 88), AV +
    ones-matmul denominators in PSUM, gpsimd broadcast + fast reciprocal
  - output projection contracts all 16 heads for the core's token eighth
"""

import os
import sys

sys.path.insert(0, "/opt/trn_rl_repo")

import ml_dtypes
import numpy as np

import concourse.mybir as mybir
import concourse.tile as tile
from concourse import bacc
from concourse.bass_utils import run_bass_kernel_spmd

F32 = mybir.dt.float32
F32R = mybir.dt.float32r
F16 = mybir.dt.float16
BF16 = mybir.dt.bfloat16
BF16_NP = ml_dtypes.bfloat16

B, S, DM = 2, 2048, 2048
H, DS, DG, DV = 16, 64, 64, 128
HPC = 2                      # heads per core
NCORES = 8
DH = DS + DG                 # 128, concat [sem|geo] per head
NSLOT = 4                    # (batch, local head) pairs per core
NKT = S // 128               # 16 key tiles per batch
NQB = S // 512               # 4 query blocks per batch
NGCH = 8                     # global 512-token chunks (2 batches x 4)
NDMK = DM // 128             # 16 contraction tiles
MASK_VAL = -10000.0
G8 = [[0, 1, 2, 3, 4, 5, 6, 7]]
# blob row offsets (AllGathered [6400, 2048] bf16)
R_X = 0                      # x: row b*2048 + dm_row, col = token
R_TRIG = 4096                # cos2 rows 4096..4159, sins 4160..4223 (f16 bits)
R_MASK = 4224                # staircase mask [128, 896]
R_WO = 4352                  # wo tiled [2048, 2048]
R_TOT = 6400
R8 = R_TOT // 8
KPH = int(os.environ.get("KPH", "3"))   # phases to build (debug/bisect)

_CACHED = {}


def _build(repeat=1):
    nc = bacc.Bacc("TRN2", target_bir_lowering=False, debug=False,
                   num_devices=NCORES)

    # per-core inputs (see _host_prep for layouts)
    cin_d = nc.dram_tensor("cin", [R8, 2048], BF16,
                           kind="ExternalInput").ap()
    wqk_d = nc.dram_tensor("wqk", [2 * HPC, 128, NDMK, 128], BF16,
                           kind="ExternalInput").ap()
    wv_d = nc.dram_tensor("wv", [128, NDMK, 256], BF16,
                          kind="ExternalInput").ap()
    out_d = nc.dram_tensor("out", [DM, 512], BF16, kind="ExternalOutput").ap()

    Exp = mybir.ActivationFunctionType.Exp

    with tile.TileContext(nc) as tc:
      for _rep in range(repeat):
        with tc.tile_pool(name="coll", bufs=1, space="DRAM") as dpool, \
             tc.tile_pool(name="consts", bufs=1) as cpool:
            # ---- collective staging: bounce -> one AllGather ----
            b_all = dpool.tile([R8, 2048], BF16)
            blob = dpool.tile([R_TOT, 2048], BF16, addr_space="Shared")
            a2a_in = dpool.tile([8, 128, HPC, 512], BF16)
            a2a_out = dpool.tile([8, 128, HPC, 512], BF16)

            nc.gpsimd.dma_start(b_all[:], cin_d[:, :])
            nc.gpsimd.collective_compute(
                "AllGather", mybir.AluOpType.bypass, replica_groups=G8,
                ins=[b_all[:].opt()], outs=[blob[:].opt()])

            # ---- constants to SBUF ----
            ones = cpool.tile([128, 1], BF16)
            nc.gpsimd.memset(ones[:], 1.0)
            # trig tables at base partition 64 so two-input DVE rope ops
            # share their operands' base partition
            trig = cpool.tile([128, 2, S], F16)
            masksB = cpool.tile([128, 896], BF16)
            nc.sync.dma_start(out=trig[64:128, 0, :],
                              in_=blob[R_TRIG:R_TRIG + 64, :].bitcast(F16))
            nc.sync.dma_start(out=trig[64:128, 1, :],
                              in_=blob[R_TRIG + 64:R_TRIG + 128,
                                       :].bitcast(F16))
            nc.sync.dma_start(out=masksB[:],
                              in_=blob[R_MASK:R_MASK + 128, 0:896])

            with tc.tile_pool(name="persist", bufs=1) as ppool:
                qcatT = ppool.tile([128, NSLOT, S], BF16)
                kcatT = ppool.tile([128, NSLOT, S], BF16)
                v_sb = ppool.tile([128, NKT, 512], BF16)

                # -------- phase 1: projections (+ rope fused) --------
                if KPH >= 1:
                  with tc.tile_pool(name="xt", bufs=NDMK + 2) as xtp, \
                       tc.tile_pool(name="wcol", bufs=5) as wcp, \
                       tc.tile_pool(name="wvst", bufs=1) as wvp, \
                       tc.tile_pool(name="rot", bufs=2) as rpool, \
                       tc.tile_pool(name="psA", bufs=4, space="PSUM") as psA:

                      wcs = []
                      for fb in range(2 * HPC):
                          wc = wcp.tile([128, NDMK, 128], BF16, tag="wc")
                          nc.sync.dma_start(out=wc[:], in_=wqk_d[fb, :, :, :])
                          wcs.append(wc)
                      wvt = wvp.tile([128, NDMK, 256], BF16)
                      nc.sync.dma_start(out=wvt[:], in_=wv_d[:, :, :])

                      for gch in range(NGCH):
                          b, tc_ = divmod(gch, 4)
                          ts_ = slice(tc_ * 512, tc_ * 512 + 512)
                          xrow = R_X + b * 2048
                          xts = []
                          for dmk in range(NDMK):
                              xt_t = xtp.tile([128, 512], BF16, tag="xt")
                              nc.sync.dma_start(
                                  out=xt_t[:],
                                  in_=blob[xrow + dmk * 128:
                                           xrow + dmk * 128 + 128, ts_])
                              xts.append(xt_t)
                          # qcat / kcat columns: 4 feature blocks of 128
                          for fb in range(2 * HPC):
                              h = fb % HPC
                              slot = b * HPC + h
                              ps_t = psA.tile([128, 512], F32, tag="ps")
                              for dmk in range(NDMK):
                                  nc.tensor.matmul(
                                      ps_t[:],
                                      wcs[fb][:, dmk, :],
                                      xts[dmk][:],
                                      start=(dmk == 0),
                                      stop=(dmk == NDMK - 1))
                              X = qcatT if fb < HPC else kcatT
                              nc.scalar.copy(X[:, slot, ts_], ps_t[:])
                              # rope this 512-token slice of the geo half
                              rot = rpool.tile([128, 512], BF16, tag="rot")
                              nc.gpsimd.tensor_copy(rot[64:96, :],
                                                    X[96:128, slot, ts_])
                              nc.gpsimd.tensor_copy(rot[96:128, :],
                                                    X[64:96, slot, ts_])
                              nc.vector.tensor_mul(rot[64:128, :],
                                                   rot[64:128, :],
                                                   trig[64:128, 1, ts_])
                              nc.vector.tensor_mul(X[64:128, slot, ts_],
                                                   X[64:128, slot, ts_],
                                                   trig[64:128, 0, ts_])
                              nc.vector.tensor_add(X[64:128, slot, ts_],
                                                   X[64:128, slot, ts_],
                                                   rot[64:128, :])
                          # v: natural layout [token, 2*dv], 4 token sub-tiles
                          for tsub in range(4):
                              tt = tc_ * 4 + tsub
                              ps_v = psA.tile([128, 256], F32, tag="psv")
                              for dmk in range(NDMK):
                                  nc.tensor.matmul(
                                      ps_v[:],
                                      xts[dmk][:,
                                               tsub * 128:tsub * 128 + 128],
                                      wvt[:, dmk, :],
                                      start=(dmk == 0),
                                      stop=(dmk == NDMK - 1))
                              nc.scalar.copy(
                                  v_sb[:, tt, b * 256:b * 256 + 256],
                                  ps_v[:])

                # -------- phase 2: attention --------
                if KPH >= 2:
                  with tc.tile_pool(name="es", bufs=2) as espool, \
                       tc.tile_pool(name="bc", bufs=3) as bcpool, \
                       tc.tile_pool(name="stg", bufs=2) as stpool, \
                       tc.tile_pool(name="psS", bufs=2, space="PSUM") as psS, \
                       tc.tile_pool(name="psO", bufs=2, space="PSUM") as psO, \
                       tc.tile_pool(name="psN", bufs=2, space="PSUM") as psN:
                    for slot in range(NSLOT):
                        b, h = divmod(slot, HPC)
                        for J in range(NQB):
                            qs_ = slice(J * 512, J * 512 + 512)
                            nkt = 4 * J + 4          # causal k-tiles
                            ps_o = psO.tile([128, 512], F32, tag="po")
                            ps_s = psN.tile([1, 512], F32, tag="pn")
                            ngrp = nkt // 2
                            for g in range(ngrp):
                                ps_sc = psS.tile([128, 1024], F32, tag="sc")
                                es = espool.tile([128, 1024], BF16, tag="es")
                                for t2 in range(2):
                                    kt = 2 * g + t2
                                    sl = slice(t2 * 512, t2 * 512 + 512)
                                    diag = kt >= 4 * J
                                    nc.tensor.matmul(
                                        ps_sc[:, sl],
                                        kcatT[:, slot,
                                              kt * 128:kt * 128 + 128],
                                        qcatT[:, slot, qs_],
                                        start=True, stop=True)
                                    if diag:
                                        t = kt - 4 * J
                                        j0 = 384 - 128 * t
                                        nc.vector.tensor_add(
                                            ps_sc[:, sl], ps_sc[:, sl],
                                            masksB[:, j0:j0 + 512])
                                nc.scalar.activation(es[:], ps_sc[:], Exp)
                                for t2 in range(2):
                                    kt = 2 * g + t2
                                    sl = slice(t2 * 512, t2 * 512 + 512)
                                    nc.tensor.matmul(
                                        ps_o[:],
                                        v_sb[:, kt,
                                             slot * 128:slot * 128 + 128],
                                        es[:, sl],
                                        start=(kt == 0),
                                        stop=(kt == nkt - 1))
                                    nc.tensor.matmul(
                                        ps_s[:], ones[:], es[:, sl],
                                        start=(kt == 0),
                                        stop=(kt == nkt - 1))
                            # normalize: broadcast sums across partitions,
                            # fast reciprocal, scale + downcast into staging
                            sums_sb = bcpool.tile([1, 512], F32, tag="ssb")
                            nc.vector.tensor_copy(sums_sb[:], ps_s[:])
                            bc = bcpool.tile([128, 512], F32, tag="bc")
                            nc.gpsimd.partition_broadcast(bc[:], sums_sb[:])
                            bcr = bcpool.tile([128, 512], F32, tag="bcr")
                            nc.vector.reciprocal_approx_fast(bcr[:], bc[:])
                            stg = stpool.tile([128, 512], BF16, tag="stg")
                            nc.vector.tensor_mul(stg[:], ps_o[:], bcr[:])
                            nc.sync.dma_start(
                                out=a2a_in[b * 4 + J, :, h, :], in_=stg[:])

            # -------- reshard: (2 heads, all tokens) -> (16 heads, eighth)
            if KPH >= 3:
              nc.gpsimd.collective_compute(
                  "AllToAll", mybir.AluOpType.bypass, replica_groups=G8,
                  ins=[a2a_in[:].opt()], outs=[a2a_out[:].opt()])

              # ------ phase 3: output projection for my token eighth ------
              with tc.tile_pool(name="att", bufs=1) as apool, \
                   tc.tile_pool(name="wo", bufs=4) as wopool, \
                   tc.tile_pool(name="ost", bufs=3) as ostp, \
                   tc.tile_pool(name="psW", bufs=4, space="PSUM") as psW:
                  att = apool.tile([128, H, 512], BF16)
                  for i in range(8):
                      for hl in range(HPC):
                          nc.sync.dma_start(out=att[:, i * HPC + hl, :],
                                            in_=a2a_out[i, :, hl, :])
                  for dmt in range(NDMK):
                      wo_t = wopool.tile([128, H, 128], BF16, tag="wo")
                      nc.sync.dma_start(
                          out=wo_t[:],
                          in_=blob[R_WO + dmt * 128:R_WO + dmt * 128 + 128,
                                   :])
                      ps_w = psW.tile([128, 512], F32, tag="pw")
                      for h in range(H):
                          nc.tensor.matmul(
                              ps_w[:],
                              wo_t[:, h, :],
                              att[:, h, :],
                              start=(h == 0), stop=(h == H - 1))
                      o_sb = ostp.tile([128, 512], BF16, tag="ost")
                      nc.scalar.copy(o_sb[:], ps_w[:])
                      nc.sync.dma_start(
                          out=out_d[dmt * 128:dmt * 128 + 128, :],
                          in_=o_sb[:])

    nc.compile()
    return nc


def _host_prep(x, wq_sem, wk_sem, wq_geo, wk_geo, wv, wo, gate_logit):
    """Build the 8 per-core input maps (all bf16 on the wire)."""
    g = 1.0 / (1.0 + np.exp(-gate_logit.astype(np.float64)))  # [H]
    sc = 1.0 / np.sqrt(DS)

    half = DG // 2
    inv_freq = 1.0 / (10000.0 ** (np.arange(half, dtype=np.float64) / half))
    ang = np.arange(S, dtype=np.float64)[:, None] * inv_freq[None, :]
    cosT = np.cos(ang).T
    sinT = np.sin(ang).T
    cos2 = np.concatenate([cosT, cosT], 0).astype(np.float16)       # [64, S]
    sins = np.concatenate([-sinT, sinT], 0).astype(np.float16)      # [64, S]

    blob = np.zeros((R_TOT, 2048), dtype=BF16_NP)
    for b in range(B):
        blob[R_X + b * 2048:R_X + (b + 1) * 2048] = \
            np.ascontiguousarray(x[b].T).astype(BF16_NP)
    # f16 bits carried through the bf16 AllGather blob
    blob[R_TRIG:R_TRIG + 128] = np.ascontiguousarray(
        np.concatenate([cos2, sins], 0)).view(np.uint16).view(BF16_NP)
    # sliding causal staircase: masks[kp, j] = 0 iff (j - 384) >= kp.
    kp = np.arange(128)[:, None]
    j = np.arange(896)[None, :]
    blob[R_MASK:R_MASK + 128, :896] = \
        np.where(j - 384 >= kp, 0.0, MASK_VAL).astype(BF16_NP)
    # wo tiled: blob_wo[dmt*128+p, h*128+c] = wo[h*128+p, dmt*128+c]
    blob[R_WO:R_WO + DM] = np.ascontiguousarray(
        wo.reshape(H, 128, NDMK, 128).transpose(2, 1, 0, 3)
        .reshape(DM, DM)).astype(BF16_NP)

    in_maps = []
    for c in range(NCORES):
        heads = [2 * c, 2 * c + 1]
        # wqk: fb 0,1 = q-cat for local heads; fb 2,3 = k-cat
        wqk = np.empty((2 * HPC, 128, NDMK, 128), dtype=np.float32)
        for i, h in enumerate(heads):
            gh = g[h]
            wq_cat = np.empty((DM, DH), dtype=np.float32)
            wq_cat[:, :DS] = wq_sem[:, h * DS:(h + 1) * DS] * \
                np.float32(gh * sc)
            wq_cat[:, DS:] = wq_geo[:, h * DG:(h + 1) * DG] * \
                np.float32((1.0 - gh) * sc)
            wk_cat = np.concatenate(
                [wk_sem[:, h * DS:(h + 1) * DS],
                 wk_geo[:, h * DG:(h + 1) * DG]], 1)
            # [p, dmk, c] = w[dmk*128+p, c]
            wqk[i] = wq_cat.reshape(NDMK, 128, DH).transpose(1, 0, 2)
            wqk[2 + i] = wk_cat.reshape(NDMK, 128, DH).transpose(1, 0, 2)
        # wv2[p, dmk, h*128+cc] = wv[dmk*128+p, (2c+h)*128+cc]
        wv_slice = wv[:, 2 * c * DV:(2 * c + 2) * DV]       # [DM, 256]
        wv_t = wv_slice.reshape(NDMK, 128, 256).transpose(1, 0, 2)
        in_maps.append({
            "cin": np.ascontiguousarray(blob[R8 * c:R8 * (c + 1)]),
            "wqk": np.ascontiguousarray(wqk).astype(BF16_NP),
            "wv": np.ascontiguousarray(wv_t).astype(BF16_NP),
        })
    return in_maps


def _run(in_maps, **kw):
    if "nc" not in _CACHED:
        _CACHED["nc"] = _build()
    return run_bass_kernel_spmd(_CACHED["nc"], in_maps,
                                core_ids=list(range(NCORES)), **kw)


def _assemble(results):
    out = np.empty((B, S, DM), dtype=np.float32)
    for c in range(NCORES):
        b, q = divmod(c, 4)
        out[b, q * 512:(q + 1) * 512, :] = \
            results[c]["out"].astype(np.float32).T
    return out


def kernel(x, wq_sem, wk_sem, wq_geo, wk_geo, wv, wo, gate_logit, **_kw):
    x = np.asarray(x, dtype=np.float32)
    wq_sem = np.asarray(wq_sem, dtype=np.float32)
    wk_sem = np.asarray(wk_sem, dtype=np.float32)
    wq_geo = np.asarray(wq_geo, dtype=np.float32)
    wk_geo = np.asarray(wk_geo, dtype=np.float32)
    wv = np.asarray(wv, dtype=np.float32)
    wo = np.asarray(wo, dtype=np.float32)
    gate_logit = np.asarray(gate_logit, dtype=np.float32)

    in_maps = _host_prep(x, wq_sem, wk_sem, wq_geo, wk_geo, wv, wo, gate_logit)
    res = _run(in_maps)
    return _assemble(res.results)
